# revision 42
# baseline (speedup 1.0000x reference)
"""Adaptive (sentinel-gated) attention kernel for one TRN2 chip (8 NeuronCores).

Data-parallel over the batch: 256 samples -> 32 per core.  Per core:
  hidden = dh @ Wh + bh                      (A=512)
  v      = enc @ Wv + bv                     (P=196 pixels, D=512)
  z      = tanh(v + hidden) @ Wav + bav
  alpha  = softmax(z)                        (over P)
  c_t    = alpha-weighted sum of enc
  s_att  = tanh(st @ Ws + bs + hidden) @ Was + bas
  beta   = softmax([z, s_att])[-1]
  c_hat  = beta * st + (1 - beta) * c_t

Layout strategy: encoder output is streamed in natural layout, converted to
bf16, and xbar-transposed so that D sits on SBUF partitions.  The big matmul
runs with Wv (bf16) stationary and transposed-x moving; tanh runs on the
scalar engine with hidden as a per-partition bias; z is a second stationary
matmul written straight into per-sample PSUM partitions; c_t is a
multiply-reduce on the vector engine against a partition-broadcast alpha.
"""

import functools

import numpy as np
import ml_dtypes

B, P, D, A = 256, 196, 512, 512
NCORES = 8
BL = B // NCORES          # 32 samples per core
BP = BL * P               # 6272 rows per core
GROUP = 8                 # max samples per softmax/ct group
NGROUP = BL // GROUP      # 4
GCOLS = GROUP * P         # 1568 columns per group
ROWT = 112                # x ingest row-tile (divides 1568, mult of 16)
NROWT = GCOLS // ROWT     # 14 row tiles per group
DT = D // 128             # 4 partition tiles of D/A


def _build(repeat=1, stage=5, trq_scalar=False, ingest=3, alt=True, csplit=0):
    import concourse.bass as bass
    import concourse.tile as tile
    from concourse import bacc, mybir
    from concourse.masks import make_identity

    f32 = mybir.dt.float32
    bf16 = mybir.dt.bfloat16
    AF = mybir.ActivationFunctionType
    ALU = mybir.AluOpType
    AX = mybir.AxisListType

    nc = bacc.Bacc("TRN2", target_bir_lowering=False, debug=False,
                   num_devices=NCORES)

    # ---- dram parameters -------------------------------------------------
    x_d = nc.dram_tensor("x", [BP, D], f32, kind="ExternalInput")
    dsT_d = nc.dram_tensor("dsT", [D, 2 * BL], f32, kind="ExternalInput")
    stn_d = nc.dram_tensor("stn", [BL, D], f32, kind="ExternalInput")
    wv_d = nc.dram_tensor("wv", [D, A], bf16, kind="ExternalInput")
    wh_d = nc.dram_tensor("wh", [D, A], f32, kind="ExternalInput")
    ws_d = nc.dram_tensor("ws", [D, A], f32, kind="ExternalInput")
    wav_d = nc.dram_tensor("wav", [A, 1], bf16, kind="ExternalInput")
    bvec_d = nc.dram_tensor("bvec", [A, 4], f32, kind="ExternalInput")
    sadj_d = nc.dram_tensor("sadj", [BL, 1], f32, kind="ExternalInput")

    sse_d = nc.dram_tensor("sse", [BL, 1], f32)
    chat_d = nc.dram_tensor("chat", [BL, D], f32, kind="ExternalOutput")
    alpha_d = nc.dram_tensor("alpha", [BL, P], f32, kind="ExternalOutput")
    beta_d = nc.dram_tensor("beta", [BL, 1], f32, kind="ExternalOutput")

    wq = None  # set below once nc engines exist
    with tile.TileContext(nc) as tc:
        with (
            tc.tile_pool(name="singles", bufs=1) as sg,
            tc.tile_pool(name="nat", bufs=3 if alt else 2) as natp,
            tc.tile_pool(name="xt", bufs=2 if alt else 3) as xtp,
            tc.tile_pool(name="tanh", bufs=2) as thp,
            tc.tile_pool(name="ab", bufs=4) as abp,
            tc.tile_pool(name="sc", bufs=2) as scp,
            tc.tile_pool(name="al", bufs=2) as alp,
            tc.tile_pool(name="st8", bufs=10) as stp,
            tc.tile_pool(name="pv", bufs=4, space="PSUM") as pvp,
            tc.tile_pool(name="zf", bufs=1, space="PSUM") as zfp,
        ):
            # ---- load weights / small tensors ---------------------------
            wq = nc.gpsimd if alt else nc.sync
            wv_sb = []
            wh_sb = []
            ws_sb = []
            wav_sb = []
            was_sb = []
            bh_sb = []
            bvh_sb = []
            bs_sb = []
            dhT_sb = []
            stT_sb = []
            bvec_sb = []
            for t in range(DT):
                r = slice(128 * t, 128 * (t + 1))
                w = sg.tile([128, 2 * BL], f32, tag=f"dsT{t}", name=f"dsT{t}")
                wq.dma_start(out=w[:], in_=dsT_d[r, :])
                dhT_sb.append(w[:, :BL])
                stT_sb.append(w[:, BL:])
                w = sg.tile([128, 4], f32, tag=f"bvec{t}", name=f"bvec{t}")
                wq.dma_start(out=w[:], in_=bvec_d[r, :])
                bvec_sb.append(w)
                was_sb.append(w[:, 0:1])
                bh_sb.append(w[:, 1:2])
                bvh_sb.append(w[:, 2:3])
                bs_sb.append(w[:, 3:4])
                w = sg.tile([128, A], f32, tag=f"wh{t}", name=f"wh{t}")
                wq.dma_start(out=w[:], in_=wh_d[r, :])
                wh_sb.append(w)
                w = sg.tile([128, A], f32, tag=f"ws{t}", name=f"ws{t}")
                wq.dma_start(out=w[:], in_=ws_d[r, :])
                ws_sb.append(w)
            for t in range(DT):
                r = slice(128 * t, 128 * (t + 1))
                w = sg.tile([128, A], bf16, tag=f"wv{t}", name=f"wv{t}")
                wq.dma_start(out=w[:], in_=wv_d[r, :])
                wv_sb.append(w)
                w = sg.tile([128, 1], bf16, tag=f"wav{t}", name=f"wav{t}")
                wq.dma_start(out=w[:], in_=wav_d[r, :])
                wav_sb.append(w)
            stn_sb = sg.tile([BL, D], f32, tag="stn")
            wq.dma_start(out=stn_sb[:], in_=stn_d[:])
            sadj_sb = sg.tile([BL, 1], f32, tag="sadj")
            wq.dma_start(out=sadj_sb[:], in_=sadj_d[:])
            ident = sg.tile([128, 128], f32, tag="ident")
            make_identity(nc, ident[:])

            # ---- phase A: hidden, sentinel ------------------------------
            hidT_sb = []   # dh@Wh + bh          (bias for sentinel path)
            hidTv_sb = []  # dh@Wh + bh + bv     (bias for main tanh)
            tanhS = []
            for a_t in range(DT):
                ca = slice(128 * a_t, 128 * (a_t + 1))
                ph = pvp.tile([128, A], f32, tag="pv", name="ph")
                for d_t in range(DT):
                    nc.tensor.matmul(ph[:, :BL], wh_sb[d_t][:, ca],
                                     dhT_sb[d_t][:], start=(d_t == 0),
                                     stop=(d_t == DT - 1))
                h1 = sg.tile([128, BL], f32, tag=f"hidT{a_t}")
                nc.scalar.activation(h1[:], ph[:, :BL], AF.Identity,
                                     bias=bh_sb[a_t][:])
                hidT_sb.append(h1)
                h2 = sg.tile([128, BL], f32, tag=f"hidTv{a_t}")
                nc.scalar.activation(h2[:], ph[:, :BL], AF.Identity,
                                     bias=bvh_sb[a_t][:])
                hidTv_sb.append(h2)

                ps = pvp.tile([128, A], f32, tag="pv", name="ps")
                for d_t in range(DT):
                    nc.tensor.matmul(ps[:, :BL], ws_sb[d_t][:, ca],
                                     stT_sb[d_t][:], start=(d_t == 0),
                                     stop=(d_t == DT - 1))
                sh = sg.tile([128, BL], f32, tag=f"sh{a_t}")
                nc.vector.tensor_add(sh[:], ps[:, :BL], h1[:])
                ts_ = sg.tile([128, BL], f32, tag=f"tanhS{a_t}")
                nc.scalar.activation(ts_[:], sh[:], AF.Tanh,
                                     bias=bs_sb[a_t][:])
                tanhS.append(ts_)

            psat = pvp.tile([BL, A], f32, tag="pv", name="psat")
            for a_t in range(DT):
                nc.tensor.matmul(psat[:, :1], tanhS[a_t][:], was_sb[a_t][:],
                                 start=(a_t == 0), stop=(a_t == DT - 1))
            satt = sg.tile([BL, 1], f32, tag="satt")
            # s_att + (bas - bav): bav never added to z, shift handled here
            nc.scalar.activation(satt[:], psat[:, :1], AF.Identity,
                                 bias=sadj_sb[:])

            # persistent accumulators
            ctT = [sg.tile([128, BL], f32, tag=f"ctT{t}", name=f"ctT{t}") for t in range(DT)]
            sums32 = sg.tile([BL, 1], f32, tag="sums32")

            # ---- main loop over groups ----------------------------------
            # smaller first/last groups: faster pipeline fill and drain
            GROUPS = [4, 8, 8, 8, 4]
            assert sum(GROUPS) == BL
            for rep in range(repeat):
                b0 = 0
                for g, gs in enumerate(GROUPS):
                    gcols = gs * P
                    nrt = gcols // ROWT
                    # ingest: load rows, convert bf16, transpose into xTg
                    xTg = xtp.tile([128, DT, GCOLS], bf16, tag="xt", name="xt")
                    xT = [xTg[:, t, :] for t in range(DT)]
                    natfs = []
                    natbs = []
                    for j in range(nrt):
                        r0 = b0 * P + j * ROWT
                        natf = natp.tile([ROWT, D], f32, tag=f"natf{j % 7}",
                                         name=f"natf{j}")
                        nc.sync.dma_start(out=natf[:], in_=x_d[r0:r0 + ROWT, :])
                        natfs.append(natf)
                    if ingest >= 2:
                        for j in range(nrt):
                            natb = natp.tile([ROWT, D], bf16,
                                             tag=f"natb{j % 7}",
                                             name=f"natb{j}")
                            on_act = {0: j % 2 == 0, 1: True,
                                      2: j % 3 != 2, 3: j % 3 == 0}[csplit]
                            if on_act:
                                nc.scalar.activation(
                                    natb[:], natfs[j][:], AF.Copy, bias=0.0)
                            else:
                                nc.vector.tensor_copy(out=natb[:],
                                                      in_=natfs[j][:])
                            natbs.append(natb)
                    trq = nc.scalar if trq_scalar else nc.sync
                    if ingest >= 3:
                        for j in range(nrt):
                            trq.dma_start(
                                out=xTg[:, :, j * ROWT:(j + 1) * ROWT],
                                in_=natbs[j][:], transpose=True)

                    # v matmul + tanh per sample
                    if stage < 2:
                        continue
                    thg = [thp.tile([128, GROUP, P], bf16, tag=f"th{t}", name=f"th{t}")
                           for t in range(DT)]
                    for rr in range(gs):
                        bb = b0 + rr
                        cols = slice(rr * P, (rr + 1) * P)
                        for a_t in range(DT):
                            ca = slice(128 * a_t, 128 * (a_t + 1))
                            pv = pvp.tile([128, A], f32, tag="pv")
                            for d_t in range(DT):
                                nc.tensor.matmul(
                                    pv[:, :P], wv_sb[d_t][:, ca],
                                    xT[d_t][:, cols], start=(d_t == 0),
                                    stop=(d_t == DT - 1))
                            nc.scalar.activation(
                                thg[a_t][:, rr, :], pv[:, :P], AF.Tanh,
                                bias=hidTv_sb[a_t][:, bb:bb + 1])

                    # z matmul + per-pair softmax + c_t (no SB->SB DMAs)
                    if stage < 3:
                        b0 += gs
                        continue
                    alef = alp.tile([1, GCOLS], f32, tag="alef", name="alef")
                    alnf = alp.tile([1, GCOLS], f32, tag="alnf", name="alnf")
                    seg = stp.tile([1, GROUP], f32, tag="seg", name="seg")
                    rig = stp.tile([1, GROUP], f32, tag="rig", name="rig")
                    zfg = zfp.tile([1, GROUP // 2, 512], f32, tag="zf",
                                   name="zfg")
                    for q in range(gs // 2):
                        for a_t in range(DT):
                            nc.tensor.matmul(
                                zfg[0:1, q, :2 * P], wav_sb[a_t][:],
                                thg[a_t][:, 2 * q:2 * q + 2, :],
                                start=(a_t == 0), stop=(a_t == DT - 1))
                    # one exp + one sum for the whole group (z bounded, f32-safe)
                    nc.scalar.activation(
                        alef[:, :gcols].rearrange("o (q c) -> o q c", q=gs // 2),
                        zfg[0:1, :gs // 2, :2 * P], AF.Exp, bias=0.0)
                    nc.vector.tensor_reduce(
                        seg[:, :gs],
                        alef[:, :gcols].rearrange("o (b p) -> o b p", b=gs),
                        AX.X, ALU.add)
                    nc.vector.reciprocal(rig[:, :gs], seg[:, :gs])
                    nc.gpsimd.dma_start(out=sse_d[b0:b0 + gs, :],
                                        in_=seg[:, :gs])
                    for rr in range(gs):
                        cols = slice(rr * P, (rr + 1) * P)
                        nc.vector.tensor_scalar_mul(alnf[:, cols],
                                                    alef[:, cols],
                                                    rig[:, rr:rr + 1])
                    nc.gpsimd.dma_start(
                        out=alpha_d[b0:b0 + gs, :],
                        in_=alnf[:, :gcols])
                    if stage < 4:
                        b0 += gs
                        continue
                    for rr in range(gs):
                        bb = b0 + rr
                        cols = slice(rr * P, (rr + 1) * P)
                        ab = abp.tile([128, P], f32, tag="ab")
                        nc.gpsimd.partition_broadcast(ab[:], alnf[:, cols])
                        for t in range(DT):
                            sc = scp.tile([128, P], f32, tag="sc")
                            nc.vector.scalar_tensor_tensor(
                                out=sc[:], in0=xT[t][:, cols], scalar=0.0,
                                in1=ab[:], op0=ALU.bypass, op1=ALU.mult,
                                accum_out=ctT[t][:, bb:bb + 1])
                    b0 += gs

            # ---- finale: beta + blend -----------------------------------
            if stage >= 5:
                nc.gpsimd.dma_start(out=sums32[:], in_=sse_d[:, :])
                es = stp.tile([BL, 1], f32, tag="es")
                nc.scalar.activation(es[:], satt[:], AF.Exp, bias=0.0)
                den = stp.tile([BL, 1], f32, tag="den")
                nc.vector.tensor_add(den[:], sums32[:], es[:])
                rden = stp.tile([BL, 1], f32, tag="rden")
                nc.vector.reciprocal(rden[:], den[:])
                bet = stp.tile([BL, 1], f32, tag="bet")
                nc.vector.tensor_mul(bet[:], es[:], rden[:])
                nc.gpsimd.dma_start(out=beta_d[:], in_=bet[:])

                ctn = pvp.tile([BL, A], f32, tag="pv", name="ctn")
                for t in range(DT):
                    nc.tensor.transpose(ctn[:, 128 * t:128 * (t + 1)],
                                        ctT[t][:], ident[:])
                dif = sg.tile([BL, D], f32, tag="dif")
                nc.vector.tensor_sub(dif[:], stn_sb[:], ctn[:])
                chat = sg.tile([BL, D], f32, tag="chat")
                nc.vector.scalar_tensor_tensor(
                    out=chat[:], in0=dif[:], scalar=bet[:], in1=ctn[:],
                    op0=ALU.mult, op1=ALU.add)
                nc.gpsimd.dma_start(out=chat_d[:], in_=chat[:])
            else:
                nc.gpsimd.dma_start(out=beta_d[:], in_=sadj_sb[:])

    nc.compile()
    return nc


@functools.lru_cache(maxsize=2)
def _built(repeat=1):
    return _build(repeat)


def _in_maps(encoder_out, decoder_hidden, st, Wv, bv, Wh, bh, Ws, bs,
             Wav, bav, Was, bas):
    f32 = np.float32
    bf16 = ml_dtypes.bfloat16
    bvec = np.stack([
        np.asarray(Was, f32), np.asarray(bh, f32),
        np.asarray(bv, f32) + np.asarray(bh, f32), np.asarray(bs, f32),
    ], axis=1)
    shared = {
        "wv": np.ascontiguousarray(Wv, f32).astype(bf16),
        "wh": np.ascontiguousarray(Wh, f32),
        "ws": np.ascontiguousarray(Ws, f32),
        "wav": np.asarray(Wav, f32).reshape(A, 1).astype(bf16),
        "bvec": np.ascontiguousarray(bvec),
    }
    sadj = np.full((BL, 1), np.float32(bas) - np.float32(bav), f32)
    maps = []
    for c in range(NCORES):
        sl = slice(BL * c, BL * (c + 1))
        m = dict(shared)
        m["x"] = np.ascontiguousarray(
            np.asarray(encoder_out[sl], f32).reshape(BP, D))
        m["dsT"] = np.ascontiguousarray(np.concatenate(
            [np.asarray(decoder_hidden[sl], f32).T,
             np.asarray(st[sl], f32).T], axis=1))
        m["stn"] = np.ascontiguousarray(np.asarray(st[sl], f32))
        m["sadj"] = sadj
        maps.append(m)
    return maps


def kernel(encoder_out, decoder_hidden, st, Wv, bv, Wh, bh, Ws, bs,
           Wav, bav, Was, bas):
    from concourse.bass_utils import run_bass_kernel_spmd

    nc = _built()
    maps = _in_maps(encoder_out, decoder_hidden, st, Wv, bv, Wh, bh, Ws, bs,
                    Wav, bav, Was, bas)
    res = run_bass_kernel_spmd(nc, maps, core_ids=list(range(NCORES)))
    chat = np.concatenate([res.results[c]["chat"] for c in range(NCORES)], 0)
    alpha = np.concatenate([res.results[c]["alpha"] for c in range(NCORES)], 0)
    beta = np.concatenate([res.results[c]["beta"] for c in range(NCORES)], 0)
    return chat, alpha, beta


# revision 45
# speedup vs baseline: 1.0186x; 1.0186x over previous
"""Adaptive (sentinel-gated) attention kernel for one TRN2 chip (8 NeuronCores).

Data-parallel over the batch: 256 samples -> 32 per core.  Per core:
  hidden = dh @ Wh + bh                      (A=512)
  v      = enc @ Wv + bv                     (P=196 pixels, D=512)
  z      = tanh(v + hidden) @ Wav + bav
  alpha  = softmax(z)                        (over P)
  c_t    = alpha-weighted sum of enc
  s_att  = tanh(st @ Ws + bs + hidden) @ Was + bas
  beta   = softmax([z, s_att])[-1]
  c_hat  = beta * st + (1 - beta) * c_t

Layout strategy: encoder output is streamed in natural layout, converted to
bf16, and xbar-transposed so that D sits on SBUF partitions.  The big matmul
runs with Wv (bf16) stationary and transposed-x moving; tanh runs on the
scalar engine with hidden as a per-partition bias; z is a second stationary
matmul written straight into per-sample PSUM partitions; c_t is a
multiply-reduce on the vector engine against a partition-broadcast alpha.
"""

import functools

import numpy as np
import ml_dtypes

B, P, D, A = 256, 196, 512, 512
NCORES = 8
BL = B // NCORES          # 32 samples per core
BP = BL * P               # 6272 rows per core
GROUP = 8                 # max samples per softmax/ct group
NGROUP = BL // GROUP      # 4
GCOLS = GROUP * P         # 1568 columns per group
ROWT = 112                # x ingest row-tile (divides 1568, mult of 16)
NROWT = GCOLS // ROWT     # 14 row tiles per group
DT = D // 128             # 4 partition tiles of D/A


def _build(repeat=1, stage=5, trq_scalar=False, ingest=3, alt=True, csplit=0, xsplit=0, gshape=0, pv5=False):
    import concourse.bass as bass
    import concourse.tile as tile
    from concourse import bacc, mybir
    from concourse.masks import make_identity

    f32 = mybir.dt.float32
    bf16 = mybir.dt.bfloat16
    AF = mybir.ActivationFunctionType
    ALU = mybir.AluOpType
    AX = mybir.AxisListType

    nc = bacc.Bacc("TRN2", target_bir_lowering=False, debug=False,
                   num_devices=NCORES)

    # ---- dram parameters -------------------------------------------------
    x_d = nc.dram_tensor("x", [BP, D], f32, kind="ExternalInput")
    dsT_d = nc.dram_tensor("dsT", [D, 2 * BL], f32, kind="ExternalInput")
    stn_d = nc.dram_tensor("stn", [BL, D], f32, kind="ExternalInput")
    wv_d = nc.dram_tensor("wv", [D, A], bf16, kind="ExternalInput")
    wh_d = nc.dram_tensor("wh", [D, A], f32, kind="ExternalInput")
    ws_d = nc.dram_tensor("ws", [D, A], f32, kind="ExternalInput")
    wav_d = nc.dram_tensor("wav", [A, 1], bf16, kind="ExternalInput")
    bvec_d = nc.dram_tensor("bvec", [A, 4], f32, kind="ExternalInput")
    sadj_d = nc.dram_tensor("sadj", [BL, 1], f32, kind="ExternalInput")

    sse_d = nc.dram_tensor("sse", [BL, 1], f32)
    chat_d = nc.dram_tensor("chat", [BL, D], f32, kind="ExternalOutput")
    alpha_d = nc.dram_tensor("alpha", [BL, P], f32, kind="ExternalOutput")
    beta_d = nc.dram_tensor("beta", [BL, 1], f32, kind="ExternalOutput")

    wq = None  # set below once nc engines exist
    with tile.TileContext(nc) as tc:
        with (
            tc.tile_pool(name="singles", bufs=1) as sg,
            tc.tile_pool(name="nat", bufs=3 if alt else 2) as natp,
            tc.tile_pool(name="xt", bufs=2 if alt else 3) as xtp,
            tc.tile_pool(name="tanh", bufs=2) as thp,
            tc.tile_pool(name="ab", bufs=4) as abp,
            tc.tile_pool(name="sc", bufs=2) as scp,
            tc.tile_pool(name="al", bufs=2) as alp,
            tc.tile_pool(name="st8", bufs=10) as stp,
            tc.tile_pool(name="pv", bufs=5 if pv5 else 4,
                         space="PSUM") as pvp,
            tc.tile_pool(name="zf", bufs=1, space="PSUM") as zfp,
        ):
            # ---- load weights / small tensors ---------------------------
            wq = nc.gpsimd if alt else nc.sync
            wv_sb = []
            wh_sb = []
            ws_sb = []
            wav_sb = []
            was_sb = []
            bh_sb = []
            bvh_sb = []
            bs_sb = []
            dhT_sb = []
            stT_sb = []
            bvec_sb = []
            for t in range(DT):
                r = slice(128 * t, 128 * (t + 1))
                w = sg.tile([128, 2 * BL], f32, tag=f"dsT{t}", name=f"dsT{t}")
                wq.dma_start(out=w[:], in_=dsT_d[r, :])
                dhT_sb.append(w[:, :BL])
                stT_sb.append(w[:, BL:])
                w = sg.tile([128, 4], f32, tag=f"bvec{t}", name=f"bvec{t}")
                wq.dma_start(out=w[:], in_=bvec_d[r, :])
                bvec_sb.append(w)
                was_sb.append(w[:, 0:1])
                bh_sb.append(w[:, 1:2])
                bvh_sb.append(w[:, 2:3])
                bs_sb.append(w[:, 3:4])
                w = sg.tile([128, A], f32, tag=f"wh{t}", name=f"wh{t}")
                wq.dma_start(out=w[:], in_=wh_d[r, :])
                wh_sb.append(w)
                w = sg.tile([128, A], f32, tag=f"ws{t}", name=f"ws{t}")
                wq.dma_start(out=w[:], in_=ws_d[r, :])
                ws_sb.append(w)
            for t in range(DT):
                r = slice(128 * t, 128 * (t + 1))
                w = sg.tile([128, A], bf16, tag=f"wv{t}", name=f"wv{t}")
                wq.dma_start(out=w[:], in_=wv_d[r, :])
                wv_sb.append(w)
                w = sg.tile([128, 1], bf16, tag=f"wav{t}", name=f"wav{t}")
                wq.dma_start(out=w[:], in_=wav_d[r, :])
                wav_sb.append(w)
            stn_sb = sg.tile([BL, D], f32, tag="stn")
            wq.dma_start(out=stn_sb[:], in_=stn_d[:])
            sadj_sb = sg.tile([BL, 1], f32, tag="sadj")
            wq.dma_start(out=sadj_sb[:], in_=sadj_d[:])
            ident = sg.tile([128, 128], f32, tag="ident")
            make_identity(nc, ident[:])

            # ---- phase A: hidden, sentinel ------------------------------
            hidT_sb = []   # dh@Wh + bh          (bias for sentinel path)
            hidTv_sb = []  # dh@Wh + bh + bv     (bias for main tanh)
            tanhS = []
            for a_t in range(DT):
                ca = slice(128 * a_t, 128 * (a_t + 1))
                ph = pvp.tile([128, A], f32, tag="pv", name="ph")
                for d_t in range(DT):
                    nc.tensor.matmul(ph[:, :BL], wh_sb[d_t][:, ca],
                                     dhT_sb[d_t][:], start=(d_t == 0),
                                     stop=(d_t == DT - 1))
                h1 = sg.tile([128, BL], f32, tag=f"hidT{a_t}")
                nc.scalar.activation(h1[:], ph[:, :BL], AF.Identity,
                                     bias=bh_sb[a_t][:])
                hidT_sb.append(h1)
                h2 = sg.tile([128, BL], f32, tag=f"hidTv{a_t}")
                nc.scalar.activation(h2[:], ph[:, :BL], AF.Identity,
                                     bias=bvh_sb[a_t][:])
                hidTv_sb.append(h2)

                ps = pvp.tile([128, A], f32, tag="pv", name="ps")
                for d_t in range(DT):
                    nc.tensor.matmul(ps[:, :BL], ws_sb[d_t][:, ca],
                                     stT_sb[d_t][:], start=(d_t == 0),
                                     stop=(d_t == DT - 1))
                sh = sg.tile([128, BL], f32, tag=f"sh{a_t}")
                nc.vector.tensor_add(sh[:], ps[:, :BL], h1[:])
                ts_ = sg.tile([128, BL], f32, tag=f"tanhS{a_t}")
                nc.scalar.activation(ts_[:], sh[:], AF.Tanh,
                                     bias=bs_sb[a_t][:])
                tanhS.append(ts_)

            psat = pvp.tile([BL, A], f32, tag="pv", name="psat")
            for a_t in range(DT):
                nc.tensor.matmul(psat[:, :1], tanhS[a_t][:], was_sb[a_t][:],
                                 start=(a_t == 0), stop=(a_t == DT - 1))
            satt = sg.tile([BL, 1], f32, tag="satt")
            # s_att + (bas - bav): bav never added to z, shift handled here
            nc.scalar.activation(satt[:], psat[:, :1], AF.Identity,
                                 bias=sadj_sb[:])

            # persistent accumulators
            ctT = [sg.tile([128, BL], f32, tag=f"ctT{t}", name=f"ctT{t}") for t in range(DT)]
            sums32 = sg.tile([BL, 1], f32, tag="sums32")

            # ---- main loop over groups ----------------------------------
            # smaller first/last groups: faster pipeline fill and drain
            GROUPS = {0: [4, 8, 8, 8, 4], 1: [8, 8, 8, 8],
                      2: [4, 4, 8, 8, 8]}[gshape]
            assert sum(GROUPS) == BL
            for rep in range(repeat):
                b0 = 0
                for g, gs in enumerate(GROUPS):
                    gcols = gs * P
                    nrt = gcols // ROWT
                    # ingest: load rows, convert bf16, transpose into xTg
                    xTg = xtp.tile([128, DT, GCOLS], bf16, tag="xt", name="xt")
                    xT = [xTg[:, t, :] for t in range(DT)]
                    natfs = []
                    natbs = []
                    for j in range(nrt):
                        r0 = b0 * P + j * ROWT
                        natf = natp.tile([ROWT, D], f32, tag=f"natf{j % 7}",
                                         name=f"natf{j}")
                        use_sw = (xsplit == 1 and j % 3 == 2) or \
                                 (xsplit == 2 and j % 2 == 1)
                        xq = nc.gpsimd if use_sw else nc.sync
                        xq.dma_start(out=natf[:], in_=x_d[r0:r0 + ROWT, :])
                        natfs.append(natf)
                    if ingest >= 2:
                        for j in range(nrt):
                            natb = natp.tile([ROWT, D], bf16,
                                             tag=f"natb{j % 7}",
                                             name=f"natb{j}")
                            on_act = {0: j % 2 == 0, 1: True,
                                      2: j % 3 != 2, 3: j % 3 == 0}[csplit]
                            if on_act:
                                nc.scalar.activation(
                                    natb[:], natfs[j][:], AF.Copy, bias=0.0)
                            else:
                                nc.vector.tensor_copy(out=natb[:],
                                                      in_=natfs[j][:])
                            natbs.append(natb)
                    trq = nc.scalar if trq_scalar else nc.sync
                    if ingest >= 3:
                        for j in range(nrt):
                            trq.dma_start(
                                out=xTg[:, :, j * ROWT:(j + 1) * ROWT],
                                in_=natbs[j][:], transpose=True)

                    # v matmul + tanh per sample
                    if stage < 2:
                        continue
                    thg = [thp.tile([128, GROUP, P], bf16, tag=f"th{t}", name=f"th{t}")
                           for t in range(DT)]
                    for rr in range(gs):
                        bb = b0 + rr
                        cols = slice(rr * P, (rr + 1) * P)
                        for a_t in range(DT):
                            ca = slice(128 * a_t, 128 * (a_t + 1))
                            pv = pvp.tile([128, A], f32, tag="pv")
                            for d_t in range(DT):
                                nc.tensor.matmul(
                                    pv[:, :P], wv_sb[d_t][:, ca],
                                    xT[d_t][:, cols], start=(d_t == 0),
                                    stop=(d_t == DT - 1))
                            nc.scalar.activation(
                                thg[a_t][:, rr, :], pv[:, :P], AF.Tanh,
                                bias=hidTv_sb[a_t][:, bb:bb + 1])

                    # z matmul + per-pair softmax + c_t (no SB->SB DMAs)
                    if stage < 3:
                        b0 += gs
                        continue
                    alef = alp.tile([1, GCOLS], f32, tag="alef", name="alef")
                    alnf = alp.tile([1, GCOLS], f32, tag="alnf", name="alnf")
                    seg = stp.tile([1, GROUP], f32, tag="seg", name="seg")
                    rig = stp.tile([1, GROUP], f32, tag="rig", name="rig")
                    nzf = 3 if pv5 else (GROUP // 2)
                    zfg = zfp.tile([1, nzf, 512], f32, tag="zf", name="zfg")
                    for q in range(gs // 2):
                        for a_t in range(DT):
                            nc.tensor.matmul(
                                zfg[0:1, q % nzf, :2 * P], wav_sb[a_t][:],
                                thg[a_t][:, 2 * q:2 * q + 2, :],
                                start=(a_t == 0), stop=(a_t == DT - 1))
                        if pv5 and (q % nzf == nzf - 1 or q == gs // 2 - 1):
                            q0 = (q // nzf) * nzf
                            nc.scalar.activation(
                                alef[:, 2 * q0 * P:2 * (q + 1) * P].rearrange(
                                    "o (q c) -> o q c", q=q - q0 + 1),
                                zfg[0:1, :q - q0 + 1, :2 * P], AF.Exp,
                                bias=0.0)
                    if not pv5:
                        # one exp + one sum for the whole group
                        nc.scalar.activation(
                            alef[:, :gcols].rearrange("o (q c) -> o q c",
                                                      q=gs // 2),
                            zfg[0:1, :gs // 2, :2 * P], AF.Exp, bias=0.0)
                    nc.vector.tensor_reduce(
                        seg[:, :gs],
                        alef[:, :gcols].rearrange("o (b p) -> o b p", b=gs),
                        AX.X, ALU.add)
                    nc.vector.reciprocal(rig[:, :gs], seg[:, :gs])
                    nc.gpsimd.dma_start(out=sse_d[b0:b0 + gs, :],
                                        in_=seg[:, :gs])
                    for rr in range(gs):
                        cols = slice(rr * P, (rr + 1) * P)
                        nc.vector.tensor_scalar_mul(alnf[:, cols],
                                                    alef[:, cols],
                                                    rig[:, rr:rr + 1])
                    nc.gpsimd.dma_start(
                        out=alpha_d[b0:b0 + gs, :],
                        in_=alnf[:, :gcols])
                    if stage < 4:
                        b0 += gs
                        continue
                    for rr in range(gs):
                        bb = b0 + rr
                        cols = slice(rr * P, (rr + 1) * P)
                        ab = abp.tile([128, P], f32, tag="ab")
                        nc.gpsimd.partition_broadcast(ab[:], alnf[:, cols])
                        for t in range(DT):
                            sc = scp.tile([128, P], f32, tag="sc")
                            nc.vector.scalar_tensor_tensor(
                                out=sc[:], in0=xT[t][:, cols], scalar=0.0,
                                in1=ab[:], op0=ALU.bypass, op1=ALU.mult,
                                accum_out=ctT[t][:, bb:bb + 1])
                    b0 += gs

            # ---- finale: beta + blend -----------------------------------
            if stage >= 5:
                nc.gpsimd.dma_start(out=sums32[:], in_=sse_d[:, :])
                es = stp.tile([BL, 1], f32, tag="es")
                nc.scalar.activation(es[:], satt[:], AF.Exp, bias=0.0)
                den = stp.tile([BL, 1], f32, tag="den")
                nc.vector.tensor_add(den[:], sums32[:], es[:])
                rden = stp.tile([BL, 1], f32, tag="rden")
                nc.vector.reciprocal(rden[:], den[:])
                bet = stp.tile([BL, 1], f32, tag="bet")
                nc.vector.tensor_mul(bet[:], es[:], rden[:])
                nc.gpsimd.dma_start(out=beta_d[:], in_=bet[:])

                ctn = pvp.tile([BL, A], f32, tag="pv", name="ctn")
                for t in range(DT):
                    nc.tensor.transpose(ctn[:, 128 * t:128 * (t + 1)],
                                        ctT[t][:], ident[:])
                dif = sg.tile([BL, D], f32, tag="dif")
                nc.vector.tensor_sub(dif[:], stn_sb[:], ctn[:])
                chat = sg.tile([BL, D], f32, tag="chat")
                nc.vector.scalar_tensor_tensor(
                    out=chat[:], in0=dif[:], scalar=bet[:], in1=ctn[:],
                    op0=ALU.mult, op1=ALU.add)
                nc.gpsimd.dma_start(out=chat_d[:], in_=chat[:])
            else:
                nc.gpsimd.dma_start(out=beta_d[:], in_=sadj_sb[:])

    nc.compile()
    return nc


@functools.lru_cache(maxsize=2)
def _built(repeat=1):
    return _build(repeat)


def _in_maps(encoder_out, decoder_hidden, st, Wv, bv, Wh, bh, Ws, bs,
             Wav, bav, Was, bas):
    f32 = np.float32
    bf16 = ml_dtypes.bfloat16
    bvec = np.stack([
        np.asarray(Was, f32), np.asarray(bh, f32),
        np.asarray(bv, f32) + np.asarray(bh, f32), np.asarray(bs, f32),
    ], axis=1)
    shared = {
        "wv": np.ascontiguousarray(Wv, f32).astype(bf16),
        "wh": np.ascontiguousarray(Wh, f32),
        "ws": np.ascontiguousarray(Ws, f32),
        "wav": np.asarray(Wav, f32).reshape(A, 1).astype(bf16),
        "bvec": np.ascontiguousarray(bvec),
    }
    sadj = np.full((BL, 1), np.float32(bas) - np.float32(bav), f32)
    maps = []
    for c in range(NCORES):
        sl = slice(BL * c, BL * (c + 1))
        m = dict(shared)
        m["x"] = np.ascontiguousarray(
            np.asarray(encoder_out[sl], f32).reshape(BP, D))
        m["dsT"] = np.ascontiguousarray(np.concatenate(
            [np.asarray(decoder_hidden[sl], f32).T,
             np.asarray(st[sl], f32).T], axis=1))
        m["stn"] = np.ascontiguousarray(np.asarray(st[sl], f32))
        m["sadj"] = sadj
        maps.append(m)
    return maps


def kernel(encoder_out, decoder_hidden, st, Wv, bv, Wh, bh, Ws, bs,
           Wav, bav, Was, bas):
    from concourse.bass_utils import run_bass_kernel_spmd

    nc = _built()
    maps = _in_maps(encoder_out, decoder_hidden, st, Wv, bv, Wh, bh, Ws, bs,
                    Wav, bav, Was, bas)
    res = run_bass_kernel_spmd(nc, maps, core_ids=list(range(NCORES)))
    chat = np.concatenate([res.results[c]["chat"] for c in range(NCORES)], 0)
    alpha = np.concatenate([res.results[c]["alpha"] for c in range(NCORES)], 0)
    beta = np.concatenate([res.results[c]["beta"] for c in range(NCORES)], 0)
    return chat, alpha, beta


# revision 50
# speedup vs baseline: 1.1699x; 1.1486x over previous
"""Adaptive (sentinel-gated) attention kernel for one TRN2 chip (8 NeuronCores).

Data-parallel over the batch: 256 samples -> 32 per core.  Per core:
  hidden = dh @ Wh + bh                      (A=512)
  v      = enc @ Wv + bv                     (P=196 pixels, D=512)
  z      = tanh(v + hidden) @ Wav + bav
  alpha  = softmax(z)                        (over P)
  c_t    = alpha-weighted sum of enc
  s_att  = tanh(st @ Ws + bs + hidden) @ Was + bas
  beta   = softmax([z, s_att])[-1]
  c_hat  = beta * st + (1 - beta) * c_t

Layout strategy: encoder output is streamed in natural layout, converted to
bf16, and xbar-transposed so that D sits on SBUF partitions.  The big matmul
runs with Wv (bf16) stationary and transposed-x moving; tanh runs on the
scalar engine with hidden as a per-partition bias; z is a second stationary
matmul written straight into per-sample PSUM partitions; c_t is a
multiply-reduce on the vector engine against a partition-broadcast alpha.
"""

import functools

import numpy as np
import ml_dtypes

B, P, D, A = 256, 196, 512, 512
NCORES = 8
BL = B // NCORES          # 32 samples per core
BP = BL * P               # 6272 rows per core
GROUP = 8                 # max samples per softmax/ct group
NGROUP = BL // GROUP      # 4
GCOLS = GROUP * P         # 1568 columns per group
ROWT = 112                # x ingest row-tile (divides 1568, mult of 16)
NROWT = GCOLS // ROWT     # 14 row tiles per group
DT = D // 128             # 4 partition tiles of D/A


def _build(repeat=1, stage=5, trq_scalar=False, ingest=3, alt=True, csplit=0, xsplit=0, gshape=0, pv5=False, halftr=False, deep=0):
    import concourse.bass as bass
    import concourse.tile as tile
    from concourse import bacc, mybir
    from concourse.masks import make_identity

    f32 = mybir.dt.float32
    bf16 = mybir.dt.bfloat16
    AF = mybir.ActivationFunctionType
    ALU = mybir.AluOpType
    AX = mybir.AxisListType

    nc = bacc.Bacc("TRN2", target_bir_lowering=False, debug=False,
                   num_devices=NCORES)

    # ---- dram parameters -------------------------------------------------
    x_d = nc.dram_tensor("x", [BP, D], f32, kind="ExternalInput")
    dsT_d = nc.dram_tensor("dsT", [D, 2 * BL], f32, kind="ExternalInput")
    stn_d = nc.dram_tensor("stn", [BL, D], f32, kind="ExternalInput")
    wv_d = nc.dram_tensor("wv", [D, A], bf16, kind="ExternalInput")
    wh_d = nc.dram_tensor("wh", [D, A], f32, kind="ExternalInput")
    ws_d = nc.dram_tensor("ws", [D, A], f32, kind="ExternalInput")
    wav_d = nc.dram_tensor("wav", [A, 1], bf16, kind="ExternalInput")
    bvec_d = nc.dram_tensor("bvec", [A, 4], f32, kind="ExternalInput")
    sadj_d = nc.dram_tensor("sadj", [BL, 1], f32, kind="ExternalInput")

    sse_d = nc.dram_tensor("sse", [BL, 1], f32)
    chat_d = nc.dram_tensor("chat", [BL, D], f32, kind="ExternalOutput")
    alpha_d = nc.dram_tensor("alpha", [BL, P], f32, kind="ExternalOutput")
    beta_d = nc.dram_tensor("beta", [BL, 1], f32, kind="ExternalOutput")

    wq = None  # set below once nc engines exist
    with tile.TileContext(nc) as tc:
        with (
            tc.tile_pool(name="singles", bufs=1) as sg,
            tc.tile_pool(name="nat", bufs=4) as natp,
            tc.tile_pool(name="xt", bufs=2 if alt else 3) as xtp,
            tc.tile_pool(name="tanh", bufs=2 if deep == -1 else 3) as thp,
            tc.tile_pool(name="ab", bufs=6 if deep == 4 else 4) as abp,
            tc.tile_pool(name="sc", bufs=3 if deep == 4 else 2) as scp,
            tc.tile_pool(name="al", bufs=2) as alp,
            tc.tile_pool(name="st8", bufs=10) as stp,
            tc.tile_pool(name="pv", bufs=5 if pv5 else 4,
                         space="PSUM") as pvp,
            tc.tile_pool(name="zf", bufs=1, space="PSUM") as zfp,
        ):
            # ---- load weights / small tensors ---------------------------
            wq = nc.gpsimd if alt else nc.sync
            wv_sb = []
            wh_sb = []
            ws_sb = []
            wav_sb = []
            was_sb = []
            bh_sb = []
            bvh_sb = []
            bs_sb = []
            dhT_sb = []
            stT_sb = []
            bvec_sb = []
            for t in range(DT):
                r = slice(128 * t, 128 * (t + 1))
                w = sg.tile([128, 2 * BL], f32, tag=f"dsT{t}", name=f"dsT{t}")
                wq.dma_start(out=w[:], in_=dsT_d[r, :])
                dhT_sb.append(w[:, :BL])
                stT_sb.append(w[:, BL:])
                w = sg.tile([128, 4], f32, tag=f"bvec{t}", name=f"bvec{t}")
                wq.dma_start(out=w[:], in_=bvec_d[r, :])
                bvec_sb.append(w)
                was_sb.append(w[:, 0:1])
                bh_sb.append(w[:, 1:2])
                bvh_sb.append(w[:, 2:3])
                bs_sb.append(w[:, 3:4])
                w = sg.tile([128, A], f32, tag=f"wh{t}", name=f"wh{t}")
                wq.dma_start(out=w[:], in_=wh_d[r, :])
                wh_sb.append(w)
                w = sg.tile([128, A], f32, tag=f"ws{t}", name=f"ws{t}")
                wq.dma_start(out=w[:], in_=ws_d[r, :])
                ws_sb.append(w)
            for t in range(DT):
                r = slice(128 * t, 128 * (t + 1))
                w = sg.tile([128, A], bf16, tag=f"wv{t}", name=f"wv{t}")
                wq.dma_start(out=w[:], in_=wv_d[r, :])
                wv_sb.append(w)
                w = sg.tile([128, 1], bf16, tag=f"wav{t}", name=f"wav{t}")
                wq.dma_start(out=w[:], in_=wav_d[r, :])
                wav_sb.append(w)
            stn_sb = sg.tile([BL, D], f32, tag="stn")
            wq.dma_start(out=stn_sb[:], in_=stn_d[:])
            sadj_sb = sg.tile([BL, 1], f32, tag="sadj")
            wq.dma_start(out=sadj_sb[:], in_=sadj_d[:])
            ident = sg.tile([128, 128], f32, tag="ident")
            make_identity(nc, ident[:])

            # ---- phase A: hidden, sentinel ------------------------------
            hidT_sb = []   # dh@Wh + bh          (bias for sentinel path)
            hidTv_sb = []  # dh@Wh + bh + bv     (bias for main tanh)
            tanhS = []
            for a_t in range(DT):
                ca = slice(128 * a_t, 128 * (a_t + 1))
                ph = pvp.tile([128, A], f32, tag="pv", name="ph")
                for d_t in range(DT):
                    nc.tensor.matmul(ph[:, :BL], wh_sb[d_t][:, ca],
                                     dhT_sb[d_t][:], start=(d_t == 0),
                                     stop=(d_t == DT - 1))
                h1 = sg.tile([128, BL], f32, tag=f"hidT{a_t}")
                nc.scalar.activation(h1[:], ph[:, :BL], AF.Identity,
                                     bias=bh_sb[a_t][:])
                hidT_sb.append(h1)
                h2 = sg.tile([128, BL], f32, tag=f"hidTv{a_t}")
                nc.scalar.activation(h2[:], ph[:, :BL], AF.Identity,
                                     bias=bvh_sb[a_t][:])
                hidTv_sb.append(h2)

                ps = pvp.tile([128, A], f32, tag="pv", name="ps")
                for d_t in range(DT):
                    nc.tensor.matmul(ps[:, :BL], ws_sb[d_t][:, ca],
                                     stT_sb[d_t][:], start=(d_t == 0),
                                     stop=(d_t == DT - 1))
                sh = sg.tile([128, BL], f32, tag=f"sh{a_t}")
                nc.vector.tensor_add(sh[:], ps[:, :BL], h1[:])
                ts_ = sg.tile([128, BL], f32, tag=f"tanhS{a_t}")
                nc.scalar.activation(ts_[:], sh[:], AF.Tanh,
                                     bias=bs_sb[a_t][:])
                tanhS.append(ts_)

            psat = pvp.tile([BL, A], f32, tag="pv", name="psat")
            for a_t in range(DT):
                nc.tensor.matmul(psat[:, :1], tanhS[a_t][:], was_sb[a_t][:],
                                 start=(a_t == 0), stop=(a_t == DT - 1))
            satt = sg.tile([BL, 1], f32, tag="satt")
            # s_att + (bas - bav): bav never added to z, shift handled here
            nc.scalar.activation(satt[:], psat[:, :1], AF.Identity,
                                 bias=sadj_sb[:])

            # persistent accumulators
            ctT = [sg.tile([128, BL], f32, tag=f"ctT{t}", name=f"ctT{t}") for t in range(DT)]
            sums32 = sg.tile([BL, 1], f32, tag="sums32")

            # ---- main loop over groups ----------------------------------
            # smaller first/last groups: faster pipeline fill and drain
            GROUPS = {0: [4, 8, 8, 8, 4], 1: [8, 8, 8, 8],
                      2: [4, 4, 8, 8, 8]}[gshape]
            assert sum(GROUPS) == BL
            for rep in range(repeat):
                b0 = 0
                for g, gs in enumerate(GROUPS):
                    gcols = gs * P
                    nrt = gcols // ROWT
                    # ingest: load rows, convert bf16, transpose into xTg
                    xTg = xtp.tile([128, DT, GCOLS], bf16, tag="xt", name="xt")
                    xT = [xTg[:, t, :] for t in range(DT)]
                    trq = nc.scalar if trq_scalar else nc.sync
                    if halftr:
                        halves = [range(0, nrt // 2), range(nrt // 2, nrt)]
                    else:
                        halves = [range(nrt)]
                    for half in halves:
                        natfs = {}
                        natbs = {}
                        for j in half:
                            r0 = b0 * P + j * ROWT
                            natf = natp.tile([ROWT, D], f32,
                                             tag=f"natf{j % 7}",
                                             name=f"natf{j}")
                            nc.sync.dma_start(out=natf[:],
                                              in_=x_d[r0:r0 + ROWT, :])
                            natfs[j] = natf
                        if ingest < 2:
                            continue
                        for j in half:
                            natb = natp.tile([ROWT, D], bf16,
                                             tag=f"natb{j % 7}",
                                             name=f"natb{j}")
                            if j % 2 == 0:
                                nc.scalar.activation(
                                    natb[:], natfs[j][:], AF.Copy, bias=0.0)
                            else:
                                nc.vector.tensor_copy(out=natb[:],
                                                      in_=natfs[j][:])
                            natbs[j] = natb
                        if ingest < 3:
                            continue
                        for j in half:
                            trq.dma_start(
                                out=xTg[:, :, j * ROWT:(j + 1) * ROWT],
                                in_=natbs[j][:], transpose=True)

                    # v matmul + tanh per sample
                    if stage < 2:
                        continue
                    thg = [thp.tile([128, GROUP, P], bf16, tag=f"th{t}", name=f"th{t}")
                           for t in range(DT)]
                    for rr in range(gs):
                        bb = b0 + rr
                        cols = slice(rr * P, (rr + 1) * P)
                        for a_t in range(DT):
                            ca = slice(128 * a_t, 128 * (a_t + 1))
                            pv = pvp.tile([128, A], f32, tag="pv")
                            for d_t in range(DT):
                                nc.tensor.matmul(
                                    pv[:, :P], wv_sb[d_t][:, ca],
                                    xT[d_t][:, cols], start=(d_t == 0),
                                    stop=(d_t == DT - 1))
                            nc.scalar.activation(
                                thg[a_t][:, rr, :], pv[:, :P], AF.Tanh,
                                bias=hidTv_sb[a_t][:, bb:bb + 1])

                    # z matmul + per-pair softmax + c_t (no SB->SB DMAs)
                    if stage < 3:
                        b0 += gs
                        continue
                    alef = alp.tile([1, GCOLS], f32, tag="alef", name="alef")
                    alnf = alp.tile([1, GCOLS], f32, tag="alnf", name="alnf")
                    seg = stp.tile([1, GROUP], f32, tag="seg", name="seg")
                    rig = stp.tile([1, GROUP], f32, tag="rig", name="rig")
                    nzf = 3 if pv5 else (GROUP // 2)
                    zfg = zfp.tile([1, nzf, 512], f32, tag="zf", name="zfg")
                    for q in range(gs // 2):
                        for a_t in range(DT):
                            nc.tensor.matmul(
                                zfg[0:1, q % nzf, :2 * P], wav_sb[a_t][:],
                                thg[a_t][:, 2 * q:2 * q + 2, :],
                                start=(a_t == 0), stop=(a_t == DT - 1))
                        if pv5 and (q % nzf == nzf - 1 or q == gs // 2 - 1):
                            q0 = (q // nzf) * nzf
                            nc.scalar.activation(
                                alef[:, 2 * q0 * P:2 * (q + 1) * P].rearrange(
                                    "o (q c) -> o q c", q=q - q0 + 1),
                                zfg[0:1, :q - q0 + 1, :2 * P], AF.Exp,
                                bias=0.0)
                    if not pv5:
                        # one exp + one sum for the whole group
                        nc.scalar.activation(
                            alef[:, :gcols].rearrange("o (q c) -> o q c",
                                                      q=gs // 2),
                            zfg[0:1, :gs // 2, :2 * P], AF.Exp, bias=0.0)
                    nc.vector.tensor_reduce(
                        seg[:, :gs],
                        alef[:, :gcols].rearrange("o (b p) -> o b p", b=gs),
                        AX.X, ALU.add)
                    nc.vector.reciprocal(rig[:, :gs], seg[:, :gs])
                    nc.gpsimd.dma_start(out=sse_d[b0:b0 + gs, :],
                                        in_=seg[:, :gs])
                    for rr in range(gs):
                        cols = slice(rr * P, (rr + 1) * P)
                        nc.vector.tensor_scalar_mul(alnf[:, cols],
                                                    alef[:, cols],
                                                    rig[:, rr:rr + 1])
                    nc.gpsimd.dma_start(
                        out=alpha_d[b0:b0 + gs, :],
                        in_=alnf[:, :gcols])
                    if stage < 4:
                        b0 += gs
                        continue
                    for rr in range(gs):
                        bb = b0 + rr
                        cols = slice(rr * P, (rr + 1) * P)
                        ab = abp.tile([128, P], f32, tag="ab")
                        nc.gpsimd.partition_broadcast(ab[:], alnf[:, cols])
                        for t in range(DT):
                            sc = scp.tile([128, P], f32, tag="sc")
                            nc.vector.scalar_tensor_tensor(
                                out=sc[:], in0=xT[t][:, cols], scalar=0.0,
                                in1=ab[:], op0=ALU.bypass, op1=ALU.mult,
                                accum_out=ctT[t][:, bb:bb + 1])
                    b0 += gs

            # ---- finale: beta + blend -----------------------------------
            if stage >= 5:
                nc.gpsimd.dma_start(out=sums32[:], in_=sse_d[:, :])
                es = stp.tile([BL, 1], f32, tag="es")
                nc.scalar.activation(es[:], satt[:], AF.Exp, bias=0.0)
                den = stp.tile([BL, 1], f32, tag="den")
                nc.vector.tensor_add(den[:], sums32[:], es[:])
                rden = stp.tile([BL, 1], f32, tag="rden")
                nc.vector.reciprocal(rden[:], den[:])
                bet = stp.tile([BL, 1], f32, tag="bet")
                nc.vector.tensor_mul(bet[:], es[:], rden[:])
                nc.gpsimd.dma_start(out=beta_d[:], in_=bet[:])

                ctn = pvp.tile([BL, A], f32, tag="pv", name="ctn")
                for t in range(DT):
                    nc.tensor.transpose(ctn[:, 128 * t:128 * (t + 1)],
                                        ctT[t][:], ident[:])
                dif = sg.tile([BL, D], f32, tag="dif")
                nc.vector.tensor_sub(dif[:], stn_sb[:], ctn[:])
                chat = sg.tile([BL, D], f32, tag="chat")
                nc.vector.scalar_tensor_tensor(
                    out=chat[:], in0=dif[:], scalar=bet[:], in1=ctn[:],
                    op0=ALU.mult, op1=ALU.add)
                nc.gpsimd.dma_start(out=chat_d[:], in_=chat[:])
            else:
                nc.gpsimd.dma_start(out=beta_d[:], in_=sadj_sb[:])

    nc.compile()
    return nc


@functools.lru_cache(maxsize=2)
def _built(repeat=1):
    return _build(repeat)


def _in_maps(encoder_out, decoder_hidden, st, Wv, bv, Wh, bh, Ws, bs,
             Wav, bav, Was, bas):
    f32 = np.float32
    bf16 = ml_dtypes.bfloat16
    bvec = np.stack([
        np.asarray(Was, f32), np.asarray(bh, f32),
        np.asarray(bv, f32) + np.asarray(bh, f32), np.asarray(bs, f32),
    ], axis=1)
    shared = {
        "wv": np.ascontiguousarray(Wv, f32).astype(bf16),
        "wh": np.ascontiguousarray(Wh, f32),
        "ws": np.ascontiguousarray(Ws, f32),
        "wav": np.asarray(Wav, f32).reshape(A, 1).astype(bf16),
        "bvec": np.ascontiguousarray(bvec),
    }
    sadj = np.full((BL, 1), np.float32(bas) - np.float32(bav), f32)
    maps = []
    for c in range(NCORES):
        sl = slice(BL * c, BL * (c + 1))
        m = dict(shared)
        m["x"] = np.ascontiguousarray(
            np.asarray(encoder_out[sl], f32).reshape(BP, D))
        m["dsT"] = np.ascontiguousarray(np.concatenate(
            [np.asarray(decoder_hidden[sl], f32).T,
             np.asarray(st[sl], f32).T], axis=1))
        m["stn"] = np.ascontiguousarray(np.asarray(st[sl], f32))
        m["sadj"] = sadj
        maps.append(m)
    return maps


def kernel(encoder_out, decoder_hidden, st, Wv, bv, Wh, bh, Ws, bs,
           Wav, bav, Was, bas):
    from concourse.bass_utils import run_bass_kernel_spmd

    nc = _built()
    maps = _in_maps(encoder_out, decoder_hidden, st, Wv, bv, Wh, bh, Ws, bs,
                    Wav, bav, Was, bas)
    res = run_bass_kernel_spmd(nc, maps, core_ids=list(range(NCORES)))
    chat = np.concatenate([res.results[c]["chat"] for c in range(NCORES)], 0)
    alpha = np.concatenate([res.results[c]["alpha"] for c in range(NCORES)], 0)
    beta = np.concatenate([res.results[c]["beta"] for c in range(NCORES)], 0)
    return chat, alpha, beta
